# revision 1
# baseline (speedup 1.0000x reference)
"""Distributed Trainium2 (8 NeuronCores) kernel for a 3-layer GraphSAGE-style GNN.

Algorithm (per layer, equivalent to the reference by linearity):
    H = x @ Wl              (each core computes rows for its own nodes, then
                             AllGather -> full bf16 H replica in local HBM)
    agg[n] = mean_{e: dst=n} H[src_e]   (dst-sharded; dma_gather pulls per-edge
                             H rows, TensorE matmuls with host-built one-hot
                             "S" matrices (inv-degree folded in) do the
                             segment-sum straight into PSUM)
    x' = leakyrelu(agg + x @ Wr + bl)   (Wr matmul accumulates into the same
                             PSUM bank; ScalarE Prelu applies bias+slope)
Then mean-pool per graph (PE transpose + one-hot graph matmul with 1/count
folded in), @ Wo + bo, AllReduce across cores.

All 8 cores run ONE SPMD program: the chunk/S-matrix schedule is canonical
across cores (max-over-cores chunk counts, min/max-over-cores column windows);
only the DATA (gather indices, S values) differs per core.
"""
import sys

sys.path.insert(0, "/opt/trn_rl_repo")

import numpy as np

import concourse.bass as bass
import concourse.bacc as bacc
import concourse.mybir as mybir
import concourse.tile as tile

BF16 = mybir.dt.bfloat16
F32 = mybir.dt.float32
I16 = mybir.dt.int16
np_bf16 = mybir.dt.np(BF16)

CORES = 8
D = 128
NEG = 0.1
N_GRAPHS = 64
N_LAYERS = 3
RANGE = 512          # nodes per PSUM accumulation bank
SRR = 4              # ranges per super-range
CALL_CHUNKS = 32     # 128-edge chunks per dma_gather call (4096 idxs)
NQUEUES = 4

_cache = {}


def _ceil(a, b):
    return (a + b - 1) // b


def _preprocess(edge_index, batch, n_nodes):
    """Build canonical schedule + per-core data arrays."""
    E = edge_index.shape[1]
    src = np.asarray(edge_index[0], dtype=np.int64)
    dst = np.asarray(edge_index[1], dtype=np.int64)
    NPC = n_nodes // CORES
    ntiles = _ceil(NPC, 128)
    NPCP = ntiles * 128
    # split the per-layer AllGather into segments of <=48 node tiles; each
    # segment has its own gathered buffer, fired as soon as its h rows are
    # ready.  Buckets are <=32k-row blocks of the segment buffers, so each
    # gather depends only on the AG segment it reads.
    segs = [(a, min(a + 48, ntiles)) for a in range(0, ntiles, 48)]
    if len(segs) > 2:
        segs = [segs[0], (segs[1][0], ntiles)]
    seg_rows = [(b - a) * 128 for a, b in segs]
    seg_blocks = []
    bucket_base = []
    nb_total = 0
    for rs in seg_rows:
        tot = CORES * rs
        nblk = max(1, _ceil(tot, 32768))
        blk = _ceil(tot, nblk)
        seg_blocks.append(blk)
        bucket_base.append(nb_total)
        nb_total += nblk
    NBKT = nb_total

    deg = np.bincount(dst, minlength=n_nodes).astype(np.float32)
    inv_deg = (1.0 / np.maximum(deg, 1.0)).astype(np.float32)

    core = dst // NPC
    dstl = (dst % NPC).astype(np.int64)
    s_rank = src // NPC
    s_l = src % NPC
    bkt = np.zeros(E, np.int64)
    idxval = np.zeros(E, np.int64)
    for i, (a, b) in enumerate(segs):
        lo, hi = a * 128, b * 128
        m = (s_l >= lo) & (s_l < min(hi, NPC))
        row = s_rank[m] * seg_rows[i] + (s_l[m] - lo)
        bkt[m] = bucket_base[i] + row // seg_blocks[i]
        idxval[m] = row % seg_blocks[i]
    nranges = _ceil(NPC, RANGE)
    nsr = _ceil(nranges, SRR)
    sr = (dstl // RANGE) // SRR

    order = np.lexsort((dstl, bkt, sr, core))
    s_src, s_dstl, s_bkt, s_sr, s_core = (
        src[order], dstl[order], bkt[order], sr[order], core[order])

    # counts per (core, sr, bucket)
    seg = ((s_core * nsr + s_sr) * NBKT + s_bkt)
    counts = np.bincount(seg, minlength=CORES * nsr * NBKT).reshape(CORES, nsr, NBKT)
    Kg = _ceil(counts.max(axis=0), 128)  # chunks per (sr, bucket), canonical
    slots_g = Kg * 128                   # [nsr, 4]
    slot_base = np.zeros((nsr, NBKT), np.int64)
    flat = slots_g.reshape(-1)
    slot_base.reshape(-1)[1:] = np.cumsum(flat)[:-1]
    TOT = int(flat.sum())
    NCHUNK = TOT // 128

    # per-edge slot position
    seg_sorted = seg  # already sorted by (core, sr, bkt) major
    # rank within segment
    seg_change = np.empty(E, bool)
    seg_change[0] = True
    seg_change[1:] = seg_sorted[1:] != seg_sorted[:-1]
    seg_start_pos = np.where(seg_change)[0]
    start_per_edge = seg_start_pos[np.cumsum(seg_change) - 1]
    rank = np.arange(E) - start_per_edge
    slot = slot_base[s_sr, s_bkt] + rank  # within-core slot

    # padded per-core arrays
    dst_pad = np.full((CORES, TOT), -1, np.int64)
    idx_pad = np.zeros((CORES, TOT), np.int16)
    dst_pad[s_core, slot] = s_dstl
    idx_pad[s_core, slot] = idxval[order].astype(np.int16)

    # canonical chunk windows: min/max real dst over all cores per chunk
    dpc = dst_pad.reshape(CORES, NCHUNK, 128)
    big = np.where(dpc < 0, np.int64(1 << 40), dpc)
    chunk_min = big.min(axis=(0, 2))
    small = np.where(dpc < 0, np.int64(-1), dpc)
    chunk_max = small.max(axis=(0, 2))
    valid_chunk = chunk_max >= 0
    chunk_min = np.where(valid_chunk, chunk_min, 0)
    chunk_max = np.where(valid_chunk, chunk_max, 0)

    # pieces: split [min, max] at RANGE boundaries; <= 1 + span/RANGE pieces
    r0 = chunk_min // RANGE
    r1 = chunk_max // RANGE
    max_rel = int((r1 - r0).max()) + 1 if NCHUNK else 1
    piece_r = np.full((NCHUNK, max_rel), -1, np.int64)
    piece_lo = np.zeros((NCHUNK, max_rel), np.int64)
    piece_W = np.zeros((NCHUNK, max_rel), np.int64)
    for rel in range(max_rel):
        r = r0 + rel
        act = valid_chunk & (r <= r1)
        lo = np.maximum(chunk_min, r * RANGE)
        hi = np.minimum(chunk_max, (r + 1) * RANGE - 1)
        piece_r[act, rel] = r[act]
        piece_lo[act, rel] = lo[act]
        piece_W[act, rel] = (hi - lo + 1)[act]
    # S column offsets, sequential over (chunk, rel)
    pw_flat = np.where(piece_r >= 0, piece_W, 0).reshape(-1)
    soff_flat = np.zeros(NCHUNK * max_rel, np.int64)
    soff_flat[1:] = np.cumsum(pw_flat)[:-1]
    piece_soff = soff_flat.reshape(NCHUNK, max_rel)
    SW = int(pw_flat.sum())

    # per-core S matrices [CORES, 128, SW]
    smat = np.zeros((CORES, 128, SW), np.float32)
    e_chunk = slot // 128
    e_row = slot % 128
    e_rel = s_dstl // RANGE - r0[e_chunk]
    e_col = piece_soff[e_chunk, e_rel] + s_dstl - piece_lo[e_chunk, e_rel]
    smat[s_core, e_row, e_col] = inv_deg[dst[order]]

    # idx param wrap: [CORES, 128, TOT//16]; partition p holds stream p%16
    idxw = idx_pad.reshape(CORES, TOT // 16, 16)  # slot-major
    idx_param = np.ascontiguousarray(
        np.tile(idxw.transpose(0, 2, 1), (1, 8, 1)))  # [CORES,128,TOT//16]

    # pooling matrices
    cnt = np.bincount(np.asarray(batch, np.int64), minlength=N_GRAPHS).astype(np.float32)
    inv_cnt = 1.0 / np.maximum(cnt, 1.0)
    gmat = np.zeros((CORES, 128, ntiles * N_GRAPHS), np.float32)
    bnp = np.asarray(batch, np.int64)
    for c in range(CORES):
        loc = bnp[c * NPC:(c + 1) * NPC]
        node = np.arange(NPC)
        t = node // 128
        p = node % 128
        gmat[c, p, t * N_GRAPHS + loc] = inv_cnt[loc]

    sched = dict(
        NPC=NPC, segs=segs, seg_rows=seg_rows, seg_blocks=seg_blocks,
        bucket_base=bucket_base, NBKT=NBKT, nranges=nranges, nsr=nsr, TOT=TOT, SW=SW,
        NCHUNK=NCHUNK, Kg=Kg, slot_base=slot_base,
        piece_r=piece_r, piece_lo=piece_lo, piece_W=piece_W,
        piece_soff=piece_soff, max_rel=max_rel, ntiles=ntiles, NPCP=NPCP,
    )
    data = dict(idx_param=idx_param, smat=smat.astype(np_bf16),
                gmat_param=gmat.astype(np_bf16))
    return sched, data


def _build_nc(sched):
    NPC = sched["NPC"]
    segs = sched["segs"]
    seg_rows = sched["seg_rows"]
    seg_blocks = sched["seg_blocks"]
    bucket_base = sched["bucket_base"]
    NBKT = sched["NBKT"]
    NSEG = len(segs)
    bkt_seg = [None] * NBKT
    bkt_blk = [None] * NBKT
    for _i in range(NSEG):
        _nb = (sched["bucket_base"][_i + 1] if _i + 1 < NSEG else NBKT) - bucket_base[_i]
        for _j in range(_nb):
            bkt_seg[bucket_base[_i] + _j] = _i
            bkt_blk[bucket_base[_i] + _j] = _j
    nranges = sched["nranges"]
    nsr = sched["nsr"]
    TOT = sched["TOT"]
    SW = sched["SW"]
    Kg = sched["Kg"]
    slot_base = sched["slot_base"]
    piece_r = sched["piece_r"]
    piece_lo = sched["piece_lo"]
    piece_W = sched["piece_W"]
    piece_soff = sched["piece_soff"]
    max_rel = sched["max_rel"]
    ntiles = sched["ntiles"]
    NPCP = sched["NPCP"]
    n_nodes = NPC * CORES

    def rwidth(r):
        return min(RANGE, NPC - r * RANGE)

    def sr_ranges(s):
        return list(range(s * SRR, min((s + 1) * SRR, nranges)))

    # per-(sr, bucket) S column spans (contiguous by construction)
    grp_scol = {}
    for s in range(nsr):
        for b in range(NBKT):
            k0 = int(slot_base[s, b]) // 128
            lo, hi = None, None
            for k in range(k0, k0 + int(Kg[s, b])):
                for rel in range(max_rel):
                    if piece_r[k, rel] >= 0 and piece_W[k, rel] > 0:
                        a = int(piece_soff[k, rel])
                        z = a + int(piece_W[k, rel])
                        lo = a if lo is None else min(lo, a)
                        hi = z if hi is None else max(hi, z)
            grp_scol[(s, b)] = (lo, hi) if lo is not None else (0, 0)

    nc = bacc.Bacc(None, target_bir_lowering=False, debug=False,
                   num_devices=CORES, num_swdge_queues=NQUEUES)
    p_x0T = nc.declare_dram_parameter("x0T", [D, NPCP], BF16, isOutput=False)
    p_idx = nc.declare_dram_parameter("idx", [128, TOT // 16], I16, isOutput=False)
    p_smat = nc.declare_dram_parameter("smat", [128, max(SW, 1)], BF16, isOutput=False)
    p_gmat = nc.declare_dram_parameter("gmat", [128, ntiles * N_GRAPHS], BF16, isOutput=False)
    p_wl = nc.declare_dram_parameter("wl", [D, N_LAYERS * D], BF16, isOutput=False)
    p_wr = nc.declare_dram_parameter("wr", [D, N_LAYERS * D], BF16, isOutput=False)
    p_bl = nc.declare_dram_parameter("bl", [D, N_LAYERS], F32, isOutput=False)
    p_wo = nc.declare_dram_parameter("wo", [D, 1], F32, isOutput=False)
    p_bo = nc.declare_dram_parameter("bo", [N_GRAPHS, 1], F32, isOutput=False)
    p_id = nc.declare_dram_parameter("ident", [D, D], BF16, isOutput=False)
    p_out = nc.declare_dram_parameter("out", [N_GRAPHS, 1], F32, isOutput=True)

    h_segs = [[nc.dram_tensor(f"h_seg{j}_{i}", [CORES * seg_rows[i], D], BF16,
                              addr_space="Shared") for i in range(NSEG)]
              for j in range(2)]
    gathers_on = {(j, i): [] for j in range(2) for i in range(NSEG)}

    with tile.TileContext(nc) as tc:
        with (
            tc.tile_pool(name="res", bufs=1) as res,
            tc.tile_pool(name="spool", bufs=3) as spool,
            tc.tile_pool(name="mpool", bufs=10) as mpool,
            tc.tile_pool(name="hpool", bufs=3) as hpool,
            tc.tile_pool(name="pagg", bufs=5, space="PSUM") as pagg,
            tc.tile_pool(name="ph", bufs=2, space="PSUM") as ph,
            tc.tile_pool(name="pmisc", bufs=1, space="PSUM") as pmisc,
            tc.tile_pool(name="dpool", bufs=2, space="DRAM") as dpool,
        ):
            # critical-path loads first: layer-0 H needs only x0T and Wl
            xT = res.tile([D, NPCP], BF16)
            nc.sync.dma_start(out=xT[:, :], in_=p_x0T[:, :])
            wl_t = res.tile([D, N_LAYERS * D], BF16)
            nc.sync.dma_start(out=wl_t[:, :], in_=p_wl[:, :])

            call_counter = 0
            pool_ps = pmisc.tile([128, N_GRAPHS], F32, tag="misc",
                                 name="pool_ps")

            def emit_h_tiles(lyr, h_loc_v, tlo, thi):
                t = tlo
                while t < thi:
                    nb = min(8, thi - t)
                    h8 = hpool.tile([128, 8, D], BF16, name="h8", tag="h8")
                    for j in range(nb):
                        tt = t + j
                        c0 = tt * 128
                        w = min(128, NPC - c0)
                        psh = ph.tile([128, D], F32, tag="ph", name="psh")
                        nc.tensor.matmul(
                            psh[0:w, :], lhsT=xT[:, c0:c0 + w],
                            rhs=wl_t[:, lyr * D:(lyr + 1) * D],
                            start=True, stop=True)
                        nc.vector.tensor_copy(h8[0:w, j, :], psh[0:w, :])
                    nc.sync.dma_start(out=h_loc_v[:, t:t + nb, :],
                                      in_=h8[:, 0:nb, :])
                    t += nb

            def emit_pool_tiles(tlo, thi):
                for t in range(tlo, thi):
                    c0 = t * 128
                    ptr = ph.tile([128, D], BF16, tag="ph", name="ptr")
                    nc.tensor.transpose(ptr[:, :], xT[:, c0:c0 + 128], id_t[:, :])
                    x3r = hpool.tile([128, D], BF16, name="x3r", tag="x3r")
                    nc.vector.tensor_copy(x3r[:, :], ptr[:, :])
                    nc.tensor.matmul(
                        pool_ps[:, :], lhsT=x3r[:, :],
                        rhs=gmat_t[:, t * N_GRAPHS:(t + 1) * N_GRAPHS],
                        start=(t == 0), stop=(t == ntiles - 1))

            def emit_ag(h_loc, buf, i):
                a, b = segs[i]
                cc = nc.gpsimd.collective_compute(
                    "AllGather", mybir.AluOpType.bypass,
                    replica_groups=[list(range(CORES))],
                    ins=[h_loc[a * 128:b * 128, :].opt()],
                    outs=[h_segs[buf][i][:, :].opt()],
                )
                for g in gathers_on[(buf, i)]:
                    bass._add_dep_helper(cc.ins, g.ins, True, "AG after old gathers")
                gathers_on[(buf, i)] = []
                return cc

            # layer-0 H upfront
            h_loc = dpool.tile([NPCP, D], BF16, name="h_loc", tag="hloc")
            h_loc_v = h_loc.rearrange("(j p) f -> p j f", p=128)
            cc_cur = []
            for i, (a, b) in enumerate(segs):
                emit_h_tiles(0, h_loc_v, a, b)
                cc_cur.append(emit_ag(h_loc, 0, i))
            # remaining resident loads, off the startup critical path
            idx_t = res.tile([128, TOT // 16], I16)
            nc.sync.dma_start(out=idx_t[:, :], in_=p_idx[:, :])
            wr_t = res.tile([D, N_LAYERS * D], BF16)
            nc.sync.dma_start(out=wr_t[:, :], in_=p_wr[:, :])
            bl_t = res.tile([D, N_LAYERS], F32)
            nc.sync.dma_start(out=bl_t[:, :], in_=p_bl[:, :])
            gmat_t = res.tile([128, ntiles * N_GRAPHS], BF16)
            nc.sync.dma_start(out=gmat_t[:, :], in_=p_gmat[:, :])
            wo_t = res.tile([D, 1], F32)
            nc.sync.dma_start(out=wo_t[:, :], in_=p_wo[:, :])
            bo_t = res.tile([N_GRAPHS, 1], F32)
            nc.sync.dma_start(out=bo_t[:, :], in_=p_bo[:, :])
            id_t = res.tile([D, D], BF16)
            nc.sync.dma_start(out=id_t[:, :], in_=p_id[:, :])

            TPS = (RANGE * SRR) // 128  # node tiles per super-range
            pending = {}
            for layer in range(N_LAYERS):
                cc_prev = list(cc_cur)
                cc_next = [None] * NSEG
                if layer < N_LAYERS - 1:
                    h_loc = dpool.tile([NPCP, D], BF16, name="h_loc", tag="hloc")
                    h_loc_v = h_loc.rearrange("(j p) f -> p j f", p=128)
                for s in range(nsr):
                    rs = sr_ranges(s)
                    aggs = {}
                    for r in rs:
                        aggs[r] = pagg.tile([128, RANGE], F32, name="agg", tag="agg")
                    # find last piece per range for stop flags
                    last_piece = {}
                    emission = []
                    for b in range(NBKT):
                        k0 = int(slot_base[s, b]) // 128
                        K = int(Kg[s, b])
                        calls = []
                        k = 0
                        while k < K:
                            nck = min(CALL_CHUNKS, K - k)
                            calls.append((k0 + k, nck))
                            k += nck
                        emission.append((b, calls))
                        for kk in range(k0, k0 + K):
                            for rel in range(max_rel):
                                r = int(piece_r[kk, rel])
                                if r >= 0 and piece_W[kk, rel] > 0:
                                    last_piece[r] = (kk, rel)
                    # Wr matmuls open the banks
                    for r in rs:
                        w = rwidth(r)
                        nc.tensor.matmul(
                            aggs[r][:, 0:w], lhsT=wr_t[:, layer * D:(layer + 1) * D],
                            rhs=xT[:, r * RANGE:r * RANGE + w],
                            start=True, stop=(r not in last_piece))
                    # gather + S matmuls
                    for b, calls in emission:
                        glo, ghi = grp_scol[(s, b)]
                        if ghi > glo:
                            s_t = spool.tile([128, ghi - glo], BF16,
                                             name="sgrp", tag="sgrp")
                            nc.sync.dma_start(out=s_t[:, :],
                                              in_=p_smat[:, glo:ghi])
                        else:
                            s_t = None
                        slo = glo
                        for (kstart, nck) in calls:
                            nidx = nck * 128
                            msg = mpool.tile([128, nck, D], BF16, name="msg")
                            colb = kstart * 8  # 128/16
                            si = bkt_seg[b]
                            blk = bkt_blk[b]
                            tot = CORES * seg_rows[si]
                            r0b = blk * seg_blocks[si]
                            r1b = min(r0b + seg_blocks[si], tot)
                            src_ap = h_segs[layer % 2][si][r0b:r1b, :]
                            if cc_prev[si] is None:
                                cc_prev[si] = pending.pop((layer % 2, si))()
                            dep_cc, ab = cc_prev[si], si
                            g = nc.gpsimd.dma_gather(
                                out_ap=msg[:, :, :],
                                in_ap=src_ap,
                                idxs_ap=idx_t[:, colb:colb + nidx // 16],
                                num_idxs=nidx, num_idxs_reg=nidx,
                                elem_size=D, single_packet=False,
                                queue_num=call_counter % NQUEUES,
                            )
                            call_counter += 1
                            bass._add_dep_helper(g.ins, dep_cc.ins, True, "gather after AG")
                            gathers_on[(layer % 2, ab)].append(g)
                            for kk in range(kstart, kstart + nck):
                                for rel in range(max_rel):
                                    r = int(piece_r[kk, rel])
                                    W = int(piece_W[kk, rel])
                                    if r < 0 or W == 0:
                                        continue
                                    soff = int(piece_soff[kk, rel]) - slo
                                    pcol = int(piece_lo[kk, rel]) - r * RANGE
                                    nc.tensor.matmul(
                                        aggs[r][:, pcol:pcol + W],
                                        lhsT=msg[:, kk - kstart, :],
                                        rhs=s_t[:, soff:soff + W],
                                        start=False,
                                        stop=(last_piece.get(r) == (kk, rel)),
                                    )
                    # finalize: leaky-relu(agg + x@Wr + bl) -> new xT columns
                    for r in rs:
                        w = rwidth(r)
                        nc.scalar.activation(
                            xT[:, r * RANGE:r * RANGE + w], aggs[r][:, 0:w],
                            mybir.ActivationFunctionType.Prelu,
                            bias=bl_t[:, layer:layer + 1], scale=1.0, alpha=NEG)
                    # pipelined next-stage work over this super-range's tiles
                    tlo = s * TPS
                    thi = min((s + 1) * TPS, ntiles)
                    if thi > tlo:
                        if layer < N_LAYERS - 1:
                            emit_h_tiles(layer + 1, h_loc_v, tlo, thi)
                            for i, (a, bb) in enumerate(segs):
                                if tlo < bb <= thi:
                                    cc_next[i] = emit_ag(h_loc, (layer + 1) % 2, i)
                        else:
                            emit_pool_tiles(tlo, thi)
                if layer < N_LAYERS - 1:
                    for i in range(NSEG):
                        if cc_next[i] is None:
                            cc_next[i] = emit_ag(h_loc, (layer + 1) % 2, i)
                    cc_cur = cc_next

            # ---- pooling epilogue ----
            pooledT = res.tile([128, N_GRAPHS], F32)
            nc.vector.tensor_copy(pooledT[:, :], pool_ps[:, :])
            fps = pmisc.tile([N_GRAPHS, 1], F32, tag="misc")
            nc.tensor.matmul(fps[:, :], lhsT=pooledT[:, :], rhs=wo_t[:, :],
                             start=True, stop=True)
            partial = res.tile([N_GRAPHS, 1], F32)
            nc.vector.tensor_copy(partial[:, :], fps[:, :])
            ar_in = dpool.tile([N_GRAPHS, 1], F32)
            nc.sync.dma_start(out=ar_in[:, :], in_=partial[:, :])
            ar_out = dpool.tile([N_GRAPHS, 1], F32)
            nc.gpsimd.collective_compute(
                "AllReduce", mybir.AluOpType.add,
                replica_groups=[list(range(CORES))],
                ins=[ar_in[:, :].opt()],
                outs=[ar_out[:, :].opt()],
            )
            summ = res.tile([N_GRAPHS, 1], F32)
            nc.sync.dma_start(out=summ[:, :], in_=ar_out[:, :])
            outt = res.tile([N_GRAPHS, 1], F32)
            nc.scalar.activation(outt[:, :], summ[:, :],
                                 mybir.ActivationFunctionType.Identity,
                                 bias=bo_t[:, 0:1], scale=1.0)
            nc.sync.dma_start(out=p_out[:, :], in_=outt[:, :])

    nc.compile()
    return nc


def _make_in_maps(node_features, Wl, bl, Wr, Wo, bo, sched, data):
    NPC = sched["NPC"]
    NPCP = sched["NPCP"]
    in_maps = []
    wl_h = np.ascontiguousarray(
        np.concatenate([np.asarray(Wl[i]) for i in range(N_LAYERS)], axis=1)
    ).astype(np_bf16)
    wr_h = np.ascontiguousarray(
        np.concatenate([np.asarray(Wr[i]) for i in range(N_LAYERS)], axis=1)
    ).astype(np_bf16)
    bl_h = np.ascontiguousarray(np.asarray(bl, np.float32).T)  # [D, L]
    wo_h = np.asarray(Wo, np.float32).reshape(D, 1)
    bo_h = np.full((N_GRAPHS, 1), np.float32(np.asarray(bo).reshape(-1)[0]))
    id_h = np.eye(D, dtype=np_bf16)
    nf = np.asarray(node_features, np.float32)
    for c in range(CORES):
        x0 = nf[c * NPC:(c + 1) * NPC]
        x0T = np.zeros((D, NPCP), np.float32)
        x0T[:, :NPC] = x0.T
        in_maps.append({
            "x0T": x0T.astype(np_bf16),
            "idx": data["idx_param"][c],
            "smat": data["smat"][c],
            "gmat": data["gmat_param"][c],
            "wl": wl_h, "wr": wr_h, "bl": bl_h,
            "wo": wo_h, "bo": bo_h, "ident": id_h,
        })
    return in_maps


def kernel(node_features, edge_index, batch, Wl, bl, Wr, Wo, bo,
           _trace=False):
    node_features = np.asarray(node_features)
    edge_index = np.asarray(edge_index)
    batch = np.asarray(batch)
    n_nodes = node_features.shape[0]

    key = (n_nodes, edge_index.shape[1],
           hash(edge_index.tobytes()) ^ hash(batch.tobytes()))
    if key in _cache:
        sched, data, nc = _cache[key]
    else:
        sched, data = _preprocess(edge_index, batch, n_nodes)
        # pooling matrices live in data via preprocess
        nc = _build_nc(sched)
        _cache.clear()
        _cache[key] = (sched, data, nc)

    in_maps = _make_in_maps(node_features, Wl, bl, Wr, Wo, bo, sched, data)

    from concourse import bass_utils
    res = bass_utils.run_bass_kernel_spmd(
        nc, in_maps, core_ids=list(range(CORES)), trace=_trace)
    out = np.asarray(res.results[0]["out"]).reshape(-1)[:N_GRAPHS]
    global last_exec_time_ns
    last_exec_time_ns = res.exec_time_ns
    return out.astype(np.float32)



# revision 3
# speedup vs baseline: 1.0470x; 1.0470x over previous
"""Distributed Trainium2 (8 NeuronCores) kernel for a 3-layer GraphSAGE-style GNN.

Algorithm (per layer, equivalent to the reference by linearity):
    H = x @ Wl              (each core computes rows for its own nodes, then
                             AllGather -> full bf16 H replica in local HBM)
    agg[n] = mean_{e: dst=n} H[src_e]   (dst-sharded; dma_gather pulls per-edge
                             H rows, TensorE matmuls with host-built one-hot
                             "S" matrices (inv-degree folded in) do the
                             segment-sum straight into PSUM)
    x' = leakyrelu(agg + x @ Wr + bl)
Then mean-pool per graph (PE transpose + one-hot graph matmul with 1/count
folded in), @ Wo + bo, AllReduce across cores.

v2 schedule: the gather stream is the bottleneck (Q7 descriptor generation,
~8.8ns/idx/queue, 4 queues).  To keep the gather queues busy across AllGather
boundaries, each layer runs in TWO PASSES over the super-ranges:
  pass 0: all gathers/matmuls whose source rows live in AG segment 0
          (+ the Wr matmul), PSUM partial drained to SBUF bf16.
  pass 1: all segment-1 gathers/matmuls; the partial is re-added via an
          identity matmul; Prelu finalizes.
Segment 0 is small (32 node tiles) so the next layer's AG-seg0 lands while
this layer's pass-1 gathers still run -> no gather stalls at layer edges.

All 8 cores run ONE SPMD program; only the DATA differs per core.
"""
import sys

sys.path.insert(0, "/opt/trn_rl_repo")

import numpy as np

import concourse.bass as bass
import concourse.bacc as bacc
import concourse.mybir as mybir
import concourse.tile as tile

BF16 = mybir.dt.bfloat16
F32 = mybir.dt.float32
I16 = mybir.dt.int16
np_bf16 = mybir.dt.np(BF16)

CORES = 8
D = 128
NEG = 0.1
N_GRAPHS = 64
N_LAYERS = 3
RANGE = 512          # nodes per PSUM accumulation bank
SRR = 4              # ranges per super-range
CALL_CHUNKS = 32     # 128-edge chunks per dma_gather call (4096 idxs)
NQUEUES = 4
SEG0_TILES = 32      # node tiles in AG segment 0 (small -> early fire)

_cache = {}


def _ceil(a, b):
    return (a + b - 1) // b


def _preprocess(edge_index, batch, n_nodes):
    """Build canonical schedule + per-core data arrays."""
    E = edge_index.shape[1]
    src = np.asarray(edge_index[0], dtype=np.int64)
    dst = np.asarray(edge_index[1], dtype=np.int64)
    NPC = n_nodes // CORES
    ntiles = _ceil(NPC, 128)
    NPCP = ntiles * 128
    s0 = min(SEG0_TILES, max(1, ntiles // 3))
    if ntiles > s0:
        segs = [(0, s0), (s0, ntiles)]
    else:
        segs = [(0, ntiles)]
    seg_rows = [(b - a) * 128 for a, b in segs]
    # buckets: <=32k-row blocks of each segment's gathered buffer (int16 idx)
    seg_blocks = []
    bucket_base = []
    nb_total = 0
    for rs in seg_rows:
        tot = CORES * rs
        nblk = max(1, _ceil(tot, 32768))
        blk = _ceil(tot, nblk)
        seg_blocks.append(blk)
        bucket_base.append(nb_total)
        nb_total += nblk
    NBKT = nb_total

    deg = np.bincount(dst, minlength=n_nodes).astype(np.float32)
    inv_deg = (1.0 / np.maximum(deg, 1.0)).astype(np.float32)

    core = dst // NPC
    dstl = (dst % NPC).astype(np.int64)
    s_rank = src // NPC
    s_l = src % NPC
    bkt = np.zeros(E, np.int64)
    idxval = np.zeros(E, np.int64)
    for i, (a, b) in enumerate(segs):
        lo, hi = a * 128, b * 128
        m = (s_l >= lo) & (s_l < min(hi, NPC))
        row = s_rank[m] * seg_rows[i] + (s_l[m] - lo)
        bkt[m] = bucket_base[i] + row // seg_blocks[i]
        idxval[m] = row % seg_blocks[i]
    nranges = _ceil(NPC, RANGE)
    nsr = _ceil(nranges, SRR)
    sr = (dstl // RANGE) // SRR

    order = np.lexsort((dstl, bkt, sr, core))
    s_src, s_dstl, s_bkt, s_sr, s_core = (
        src[order], dstl[order], bkt[order], sr[order], core[order])

    # counts per (core, sr, bucket)
    seg = ((s_core * nsr + s_sr) * NBKT + s_bkt)
    counts = np.bincount(seg, minlength=CORES * nsr * NBKT).reshape(CORES, nsr, NBKT)
    Kg = _ceil(counts.max(axis=0), 128)  # chunks per (sr, bucket), canonical
    slots_g = Kg * 128                   # [nsr, NBKT]
    slot_base = np.zeros((nsr, NBKT), np.int64)
    flat = slots_g.reshape(-1)
    slot_base.reshape(-1)[1:] = np.cumsum(flat)[:-1]
    TOT = int(flat.sum())
    NCHUNK = TOT // 128

    # per-edge slot position
    seg_sorted = seg  # already sorted by (core, sr, bkt) major
    seg_change = np.empty(E, bool)
    seg_change[0] = True
    seg_change[1:] = seg_sorted[1:] != seg_sorted[:-1]
    seg_start_pos = np.where(seg_change)[0]
    start_per_edge = seg_start_pos[np.cumsum(seg_change) - 1]
    rank = np.arange(E) - start_per_edge
    slot = slot_base[s_sr, s_bkt] + rank  # within-core slot

    # padded per-core arrays
    dst_pad = np.full((CORES, TOT), -1, np.int64)
    idx_pad = np.zeros((CORES, TOT), np.int16)
    dst_pad[s_core, slot] = s_dstl
    idx_pad[s_core, slot] = idxval[order].astype(np.int16)

    # canonical chunk windows: min/max real dst over all cores per chunk
    dpc = dst_pad.reshape(CORES, NCHUNK, 128)
    big = np.where(dpc < 0, np.int64(1 << 40), dpc)
    chunk_min = big.min(axis=(0, 2))
    small = np.where(dpc < 0, np.int64(-1), dpc)
    chunk_max = small.max(axis=(0, 2))
    valid_chunk = chunk_max >= 0
    chunk_min = np.where(valid_chunk, chunk_min, 0)
    chunk_max = np.where(valid_chunk, chunk_max, 0)

    # pieces: split [min, max] at RANGE boundaries
    r0 = chunk_min // RANGE
    r1 = chunk_max // RANGE
    max_rel = int((r1 - r0).max()) + 1 if NCHUNK else 1
    piece_r = np.full((NCHUNK, max_rel), -1, np.int64)
    piece_lo = np.zeros((NCHUNK, max_rel), np.int64)
    piece_W = np.zeros((NCHUNK, max_rel), np.int64)
    for rel in range(max_rel):
        r = r0 + rel
        act = valid_chunk & (r <= r1)
        lo = np.maximum(chunk_min, r * RANGE)
        hi = np.minimum(chunk_max, (r + 1) * RANGE - 1)
        piece_r[act, rel] = r[act]
        piece_lo[act, rel] = lo[act]
        piece_W[act, rel] = (hi - lo + 1)[act]
    pw_flat = np.where(piece_r >= 0, piece_W, 0).reshape(-1)
    soff_flat = np.zeros(NCHUNK * max_rel, np.int64)
    soff_flat[1:] = np.cumsum(pw_flat)[:-1]
    piece_soff = soff_flat.reshape(NCHUNK, max_rel)
    SW = int(pw_flat.sum())

    # per-core S matrices [CORES, 128, SW]
    smat = np.zeros((CORES, 128, SW), np.float32)
    e_chunk = slot // 128
    e_row = slot % 128
    e_rel = s_dstl // RANGE - r0[e_chunk]
    e_col = piece_soff[e_chunk, e_rel] + s_dstl - piece_lo[e_chunk, e_rel]
    smat[s_core, e_row, e_col] = inv_deg[dst[order]]

    # idx param wrap: [CORES, 128, TOT//16]; partition p holds stream p%16
    idxw = idx_pad.reshape(CORES, TOT // 16, 16)  # slot-major
    idx_param = np.ascontiguousarray(
        np.tile(idxw.transpose(0, 2, 1), (1, 8, 1)))  # [CORES,128,TOT//16]

    # pooling matrices
    cnt = np.bincount(np.asarray(batch, np.int64), minlength=N_GRAPHS).astype(np.float32)
    inv_cnt = 1.0 / np.maximum(cnt, 1.0)
    gmat = np.zeros((CORES, 128, ntiles * N_GRAPHS), np.float32)
    bnp = np.asarray(batch, np.int64)
    for c in range(CORES):
        loc = bnp[c * NPC:(c + 1) * NPC]
        node = np.arange(NPC)
        t = node // 128
        p = node % 128
        gmat[c, p, t * N_GRAPHS + loc] = inv_cnt[loc]

    sched = dict(
        NPC=NPC, segs=segs, seg_rows=seg_rows, seg_blocks=seg_blocks,
        bucket_base=bucket_base, NBKT=NBKT, nranges=nranges, nsr=nsr, TOT=TOT, SW=SW,
        NCHUNK=NCHUNK, Kg=Kg, slot_base=slot_base,
        piece_r=piece_r, piece_lo=piece_lo, piece_W=piece_W,
        piece_soff=piece_soff, max_rel=max_rel, ntiles=ntiles, NPCP=NPCP,
    )
    data = dict(idx_param=idx_param, smat=smat.astype(np_bf16),
                gmat_param=gmat.astype(np_bf16))
    return sched, data


def _build_nc(sched):
    NPC = sched["NPC"]
    segs = sched["segs"]
    seg_rows = sched["seg_rows"]
    seg_blocks = sched["seg_blocks"]
    bucket_base = sched["bucket_base"]
    NBKT = sched["NBKT"]
    NSEG = len(segs)
    bkt_seg = [None] * NBKT
    bkt_blk = [None] * NBKT
    for _i in range(NSEG):
        _nb = (bucket_base[_i + 1] if _i + 1 < NSEG else NBKT) - bucket_base[_i]
        for _j in range(_nb):
            bkt_seg[bucket_base[_i] + _j] = _i
            bkt_blk[bucket_base[_i] + _j] = _j
    seg_buckets = [[b for b in range(NBKT) if bkt_seg[b] == i] for i in range(NSEG)]
    nranges = sched["nranges"]
    nsr = sched["nsr"]
    TOT = sched["TOT"]
    SW = sched["SW"]
    Kg = sched["Kg"]
    slot_base = sched["slot_base"]
    piece_r = sched["piece_r"]
    piece_lo = sched["piece_lo"]
    piece_W = sched["piece_W"]
    piece_soff = sched["piece_soff"]
    max_rel = sched["max_rel"]
    ntiles = sched["ntiles"]
    NPCP = sched["NPCP"]

    def rwidth(r):
        return min(RANGE, NPC - r * RANGE)

    def sr_ranges(s):
        return list(range(s * SRR, min((s + 1) * SRR, nranges)))

    # per-(sr, bucket) S column spans (contiguous by construction)
    grp_scol = {}
    for s in range(nsr):
        for b in range(NBKT):
            k0 = int(slot_base[s, b]) // 128
            lo, hi = None, None
            for k in range(k0, k0 + int(Kg[s, b])):
                for rel in range(max_rel):
                    if piece_r[k, rel] >= 0 and piece_W[k, rel] > 0:
                        a = int(piece_soff[k, rel])
                        z = a + int(piece_W[k, rel])
                        lo = a if lo is None else min(lo, a)
                        hi = z if hi is None else max(hi, z)
            grp_scol[(s, b)] = (lo, hi) if lo is not None else (0, 0)

    # pieces per (sr, pass, range): for start/stop flags
    def pass_pieces(s, seg_i):
        """{r: [(kk, rel), ...]} for pieces of chunks in seg_i's buckets."""
        out = {}
        for b in seg_buckets[seg_i]:
            k0 = int(slot_base[s, b]) // 128
            for kk in range(k0, k0 + int(Kg[s, b])):
                for rel in range(max_rel):
                    r = int(piece_r[kk, rel])
                    if r >= 0 and piece_W[kk, rel] > 0:
                        out.setdefault(r, []).append((kk, rel))
        return out

    nc = bacc.Bacc(None, target_bir_lowering=False, debug=False,
                   num_devices=CORES, num_swdge_queues=NQUEUES)
    p_x0T = nc.declare_dram_parameter("x0T", [D, NPCP], BF16, isOutput=False)
    p_idx = nc.declare_dram_parameter("idx", [128, TOT // 16], I16, isOutput=False)
    p_smat = nc.declare_dram_parameter("smat", [128, max(SW, 1)], BF16, isOutput=False)
    p_gmat = nc.declare_dram_parameter("gmat", [128, ntiles * N_GRAPHS], BF16, isOutput=False)
    p_wl = nc.declare_dram_parameter("wl", [D, N_LAYERS * D], BF16, isOutput=False)
    p_wr = nc.declare_dram_parameter("wr", [D, N_LAYERS * D], BF16, isOutput=False)
    p_bl = nc.declare_dram_parameter("bl", [D, N_LAYERS], F32, isOutput=False)
    p_wo = nc.declare_dram_parameter("wo", [D, 1], F32, isOutput=False)
    p_bo = nc.declare_dram_parameter("bo", [N_GRAPHS, 1], F32, isOutput=False)
    p_id = nc.declare_dram_parameter("ident", [D, D], BF16, isOutput=False)
    p_out = nc.declare_dram_parameter("out", [N_GRAPHS, 1], F32, isOutput=True)

    h_segs = [[nc.dram_tensor(f"h_seg{j}_{i}", [CORES * seg_rows[i], D], BF16,
                              addr_space="Shared") for i in range(NSEG)]
              for j in range(2)]
    gathers_on = {(j, i): [] for j in range(2) for i in range(NSEG)}

    with tile.TileContext(nc) as tc:
        with (
            tc.tile_pool(name="res", bufs=1) as res,
            tc.tile_pool(name="spool", bufs=3) as spool,
            tc.tile_pool(name="mpool", bufs=6) as mpool,
            tc.tile_pool(name="hpool", bufs=3) as hpool,
            tc.tile_pool(name="pagg", bufs=5, space="PSUM") as pagg,
            tc.tile_pool(name="ph", bufs=2, space="PSUM") as ph,
            tc.tile_pool(name="pmisc", bufs=1, space="PSUM") as pmisc,
            tc.tile_pool(name="dpool", bufs=2, space="DRAM") as dpool,
        ):
            # critical-path loads first: layer-0 H seg0 needs xT seg0 cols + Wl
            seg0_cols = segs[0][1] * 128
            xT = res.tile([D, NPCP], BF16)
            nc.sync.dma_start(out=xT[:, 0:seg0_cols], in_=p_x0T[:, 0:seg0_cols])
            wl_t = res.tile([D, N_LAYERS * D], BF16)
            nc.sync.dma_start(out=wl_t[:, :], in_=p_wl[:, :])

            call_counter = 0
            pool_ps = pmisc.tile([128, N_GRAPHS], F32, tag="misc",
                                 name="pool_ps")
            # bf16 partial (Wr + seg0 agg) per node column, resident
            part_t = res.tile([D, NPCP], BF16)

            def emit_h_tiles(lyr, h_loc_v, tlo, thi):
                t = tlo
                while t < thi:
                    nb = min(8, thi - t)
                    h8 = hpool.tile([128, 8, D], BF16, name="h8", tag="h8")
                    for j in range(nb):
                        tt = t + j
                        c0 = tt * 128
                        w = min(128, NPC - c0)
                        psh = ph.tile([128, D], F32, tag="ph", name="psh")
                        nc.tensor.matmul(
                            psh[0:w, :], lhsT=xT[:, c0:c0 + w],
                            rhs=wl_t[:, lyr * D:(lyr + 1) * D],
                            start=True, stop=True)
                        nc.vector.tensor_copy(h8[0:w, j, :], psh[0:w, :])
                    nc.sync.dma_start(out=h_loc_v[:, t:t + nb, :],
                                      in_=h8[:, 0:nb, :])
                    t += nb

            def emit_pool_tiles(tlo, thi):
                for t in range(tlo, thi):
                    c0 = t * 128
                    ptr = ph.tile([128, D], BF16, tag="ph", name="ptr")
                    nc.tensor.transpose(ptr[:, :], xT[:, c0:c0 + 128], id_t[:, :])
                    x3r = hpool.tile([128, D], BF16, name="x3r", tag="x3r")
                    nc.vector.tensor_copy(x3r[:, :], ptr[:, :])
                    nc.tensor.matmul(
                        pool_ps[:, :], lhsT=x3r[:, :],
                        rhs=gmat_t[:, t * N_GRAPHS:(t + 1) * N_GRAPHS],
                        start=(t == 0), stop=(t == ntiles - 1))

            def emit_ag(h_loc, buf, i):
                a, b = segs[i]
                cc = nc.gpsimd.collective_compute(
                    "AllGather", mybir.AluOpType.bypass,
                    replica_groups=[list(range(CORES))],
                    ins=[h_loc[a * 128:b * 128, :].opt()],
                    outs=[h_segs[buf][i][:, :].opt()],
                )
                for g in gathers_on[(buf, i)]:
                    bass._add_dep_helper(cc.ins, g.ins, True, "AG after old gathers")
                gathers_on[(buf, i)] = []
                return cc

            def emit_group(s, b, layer, cc_cur, last_piece, first_piece, aggs):
                """Gathers + S matmuls for one (sr, bucket) group."""
                nonlocal call_counter
                k0 = int(slot_base[s, b]) // 128
                K = int(Kg[s, b])
                if K == 0:
                    return
                glo, ghi = grp_scol[(s, b)]
                if ghi > glo:
                    s_t = spool.tile([128, ghi - glo], BF16,
                                     name="sgrp", tag="sgrp")
                    nc.sync.dma_start(out=s_t[:, :], in_=p_smat[:, glo:ghi])
                else:
                    s_t = None
                slo = glo
                k = 0
                while k < K:
                    nck = min(CALL_CHUNKS, K - k)
                    kstart = k0 + k
                    nidx = nck * 128
                    msg = mpool.tile([128, nck, D], BF16, name="msg")
                    colb = kstart * 8  # 128/16
                    si = bkt_seg[b]
                    blk = bkt_blk[b]
                    tot = CORES * seg_rows[si]
                    r0b = blk * seg_blocks[si]
                    r1b = min(r0b + seg_blocks[si], tot)
                    src_ap = h_segs[layer % 2][si][r0b:r1b, :]
                    g = nc.gpsimd.dma_gather(
                        out_ap=msg[:, :, :],
                        in_ap=src_ap,
                        idxs_ap=idx_t[:, colb:colb + nidx // 16],
                        num_idxs=nidx, num_idxs_reg=nidx,
                        elem_size=D, single_packet=False,
                        queue_num=call_counter % NQUEUES,
                    )
                    call_counter += 1
                    bass._add_dep_helper(g.ins, cc_cur[si].ins, True, "gather after AG")
                    gathers_on[(layer % 2, si)].append(g)
                    for kk in range(kstart, kstart + nck):
                        for rel in range(max_rel):
                            r = int(piece_r[kk, rel])
                            W = int(piece_W[kk, rel])
                            if r < 0 or W == 0:
                                continue
                            soff = int(piece_soff[kk, rel]) - slo
                            pcol = int(piece_lo[kk, rel]) - r * RANGE
                            nc.tensor.matmul(
                                aggs[r][:, pcol:pcol + W],
                                lhsT=msg[:, kk - kstart, :],
                                rhs=s_t[:, soff:soff + W],
                                start=(first_piece.get(r) == (kk, rel)),
                                stop=(last_piece.get(r) == (kk, rel)),
                            )
                    k += nck

            # ---- layer-0 head: seg0 H + AG0 first ----
            h_loc = dpool.tile([NPCP, D], BF16, name="h_loc", tag="hloc")
            h_loc_v = h_loc.rearrange("(j p) f -> p j f", p=128)
            cc_cur = [None] * NSEG
            emit_h_tiles(0, h_loc_v, segs[0][0], segs[0][1])
            cc_cur[0] = emit_ag(h_loc, 0, 0)
            # rest of resident loads (off the startup critical path)
            if NSEG > 1:
                nc.sync.dma_start(out=xT[:, seg0_cols:], in_=p_x0T[:, seg0_cols:])
            idx_t = res.tile([128, TOT // 16], I16)
            nc.sync.dma_start(out=idx_t[:, :], in_=p_idx[:, :])
            wr_t = res.tile([D, N_LAYERS * D], BF16)
            nc.sync.dma_start(out=wr_t[:, :], in_=p_wr[:, :])
            bl_t = res.tile([D, N_LAYERS], F32)
            nc.sync.dma_start(out=bl_t[:, :], in_=p_bl[:, :])
            gmat_t = res.tile([128, ntiles * N_GRAPHS], BF16)
            nc.sync.dma_start(out=gmat_t[:, :], in_=p_gmat[:, :])
            wo_t = res.tile([D, 1], F32)
            nc.sync.dma_start(out=wo_t[:, :], in_=p_wo[:, :])
            bo_t = res.tile([N_GRAPHS, 1], F32)
            nc.sync.dma_start(out=bo_t[:, :], in_=p_bo[:, :])
            id_t = res.tile([D, D], BF16)
            nc.sync.dma_start(out=id_t[:, :], in_=p_id[:, :])
            if NSEG > 1:
                emit_h_tiles(0, h_loc_v, segs[1][0], segs[1][1])
                cc_cur[1] = emit_ag(h_loc, 0, 1)

            TPS = (RANGE * SRR) // 128  # node tiles per super-range
            for layer in range(N_LAYERS):
                cc_next = [None] * NSEG
                if layer < N_LAYERS - 1:
                    h_loc = dpool.tile([NPCP, D], BF16, name="h_loc", tag="hloc")
                    h_loc_v = h_loc.rearrange("(j p) f -> p j f", p=128)

                # ---- PASS 0: Wr + segment-0 groups -> bf16 partial ----
                p0_pieces = [pass_pieces(s, 0) for s in range(nsr)]
                p1_pieces = ([pass_pieces(s, 1) for s in range(nsr)]
                             if NSEG > 1 else [{} for _ in range(nsr)])
                p0_aggs = {}
                for s in range(nsr):
                    rs = sr_ranges(s)
                    aggs = {r: pagg.tile([128, RANGE], F32, name="agg", tag="agg")
                            for r in rs}
                    p0_aggs[s] = aggs
                    pieces = p0_pieces[s]
                    last_piece = {r: v[-1] for r, v in pieces.items()}
                    for r in rs:
                        w = rwidth(r)
                        nc.tensor.matmul(
                            aggs[r][:, 0:w], lhsT=wr_t[:, layer * D:(layer + 1) * D],
                            rhs=xT[:, r * RANGE:r * RANGE + w],
                            start=True, stop=(r not in last_piece))
                    for b in seg_buckets[0]:
                        emit_group(s, b, layer, cc_cur, last_piece, {}, aggs)
                    # drain partial (f32 PSUM -> bf16 SBUF)
                    for r in rs:
                        w = rwidth(r)
                        nc.vector.tensor_copy(
                            part_t[:, r * RANGE:r * RANGE + w], aggs[r][:, 0:w])

                # ---- PASS 1: re-add partial + segment-1 groups -> Prelu ----
                for s in range(nsr):
                    rs = sr_ranges(s)
                    aggs = {r: pagg.tile([128, RANGE], F32, name="agg2", tag="agg")
                            for r in rs}
                    pieces = p1_pieces[s]
                    last_piece = {r: v[-1] for r, v in pieces.items()}
                    for r in rs:
                        w = rwidth(r)
                        nc.tensor.matmul(
                            aggs[r][:, 0:w], lhsT=id_t[:, :],
                            rhs=part_t[:, r * RANGE:r * RANGE + w],
                            start=True, stop=(r not in last_piece))
                    if NSEG > 1:
                        for b in seg_buckets[1]:
                            emit_group(s, b, layer, cc_cur, last_piece, {}, aggs)
                    # finalize: leaky-relu(agg + x@Wr + bl) -> new xT columns
                    for r in rs:
                        w = rwidth(r)
                        nc.scalar.activation(
                            xT[:, r * RANGE:r * RANGE + w], aggs[r][:, 0:w],
                            mybir.ActivationFunctionType.Prelu,
                            bias=bl_t[:, layer:layer + 1], scale=1.0, alpha=NEG)
                    # pipelined next-stage work over this super-range's tiles
                    tlo = s * TPS
                    thi = min((s + 1) * TPS, ntiles)
                    if thi > tlo:
                        if layer < N_LAYERS - 1:
                            emit_h_tiles(layer + 1, h_loc_v, tlo, thi)
                            for i, (a, bb) in enumerate(segs):
                                if tlo < bb <= thi:
                                    cc_next[i] = emit_ag(h_loc, (layer + 1) % 2, i)
                        else:
                            emit_pool_tiles(tlo, thi)
                if layer < N_LAYERS - 1:
                    for i in range(NSEG):
                        if cc_next[i] is None:
                            cc_next[i] = emit_ag(h_loc, (layer + 1) % 2, i)
                    cc_cur = cc_next

            # ---- pooling epilogue ----
            pooledT = res.tile([128, N_GRAPHS], F32)
            nc.vector.tensor_copy(pooledT[:, :], pool_ps[:, :])
            fps = pmisc.tile([N_GRAPHS, 1], F32, tag="misc")
            nc.tensor.matmul(fps[:, :], lhsT=pooledT[:, :], rhs=wo_t[:, :],
                             start=True, stop=True)
            partial = res.tile([N_GRAPHS, 1], F32)
            nc.vector.tensor_copy(partial[:, :], fps[:, :])
            ar_in = dpool.tile([N_GRAPHS, 1], F32)
            nc.sync.dma_start(out=ar_in[:, :], in_=partial[:, :])
            ar_out = dpool.tile([N_GRAPHS, 1], F32)
            nc.gpsimd.collective_compute(
                "AllReduce", mybir.AluOpType.add,
                replica_groups=[list(range(CORES))],
                ins=[ar_in[:, :].opt()],
                outs=[ar_out[:, :].opt()],
            )
            summ = res.tile([N_GRAPHS, 1], F32)
            nc.sync.dma_start(out=summ[:, :], in_=ar_out[:, :])
            outt = res.tile([N_GRAPHS, 1], F32)
            nc.scalar.activation(outt[:, :], summ[:, :],
                                 mybir.ActivationFunctionType.Identity,
                                 bias=bo_t[:, 0:1], scale=1.0)
            nc.sync.dma_start(out=p_out[:, :], in_=outt[:, :])

    nc.compile()
    return nc


def _make_in_maps(node_features, Wl, bl, Wr, Wo, bo, sched, data):
    NPC = sched["NPC"]
    NPCP = sched["NPCP"]
    in_maps = []
    wl_h = np.ascontiguousarray(
        np.concatenate([np.asarray(Wl[i]) for i in range(N_LAYERS)], axis=1)
    ).astype(np_bf16)
    wr_h = np.ascontiguousarray(
        np.concatenate([np.asarray(Wr[i]) for i in range(N_LAYERS)], axis=1)
    ).astype(np_bf16)
    bl_h = np.ascontiguousarray(np.asarray(bl, np.float32).T)  # [D, L]
    wo_h = np.asarray(Wo, np.float32).reshape(D, 1)
    bo_h = np.full((N_GRAPHS, 1), np.float32(np.asarray(bo).reshape(-1)[0]))
    id_h = np.eye(D, dtype=np_bf16)
    nf = np.asarray(node_features, np.float32)
    for c in range(CORES):
        x0 = nf[c * NPC:(c + 1) * NPC]
        x0T = np.zeros((D, NPCP), np.float32)
        x0T[:, :NPC] = x0.T
        in_maps.append({
            "x0T": x0T.astype(np_bf16),
            "idx": data["idx_param"][c],
            "smat": data["smat"][c],
            "gmat": data["gmat_param"][c],
            "wl": wl_h, "wr": wr_h, "bl": bl_h,
            "wo": wo_h, "bo": bo_h, "ident": id_h,
        })
    return in_maps


def kernel(node_features, edge_index, batch, Wl, bl, Wr, Wo, bo,
           _trace=False):
    node_features = np.asarray(node_features)
    edge_index = np.asarray(edge_index)
    batch = np.asarray(batch)
    n_nodes = node_features.shape[0]

    key = (n_nodes, edge_index.shape[1],
           hash(edge_index.tobytes()) ^ hash(batch.tobytes()))
    if key in _cache:
        sched, data, nc = _cache[key]
    else:
        sched, data = _preprocess(edge_index, batch, n_nodes)
        nc = _build_nc(sched)
        _cache.clear()
        _cache[key] = (sched, data, nc)

    in_maps = _make_in_maps(node_features, Wl, bl, Wr, Wo, bo, sched, data)

    from concourse import bass_utils
    res = bass_utils.run_bass_kernel_spmd(
        nc, in_maps, core_ids=list(range(CORES)), trace=_trace)
    out = np.asarray(res.results[0]["out"]).reshape(-1)[:N_GRAPHS]
    global last_exec_time_ns
    last_exec_time_ns = res.exec_time_ns
    return out.astype(np.float32)


# revision 14
# speedup vs baseline: 6.2228x; 5.9437x over previous
"""Distributed Trainium2 (8 NeuronCores) kernel for a 3-layer GraphSAGE-style GNN.

Algorithm (per layer, equivalent to the reference by linearity):
    H = x @ Wl              (each core computes rows for its own nodes, then
                             AllGather -> full bf16 H replica in local HBM)
    agg[n] = mean_{e: dst=n} H[src_e]   (dst-sharded; dma_gather pulls per-edge
                             H rows, TensorE matmuls with host-built one-hot
                             "S" matrices (inv-degree folded in) do the
                             segment-sum straight into PSUM)
    x' = leakyrelu(agg + x @ Wr + bl)
Then mean-pool per graph (PE transpose + one-hot graph matmul with 1/count
folded in), @ Wo + bo, AllReduce across cores.

v2 schedule: the gather stream is the bottleneck (Q7 descriptor generation,
~8.8ns/idx/queue, 4 queues).  To keep the gather queues busy across AllGather
boundaries, each layer runs in TWO PASSES over the super-ranges:
  pass 0: all gathers/matmuls whose source rows live in AG segment 0
          (+ the Wr matmul), PSUM partial drained to SBUF bf16.
  pass 1: all segment-1 gathers/matmuls; the partial is re-added via an
          identity matmul; Prelu finalizes.
Segment 0 is small (32 node tiles) so the next layer's AG-seg0 lands while
this layer's pass-1 gathers still run -> no gather stalls at layer edges.

All 8 cores run ONE SPMD program; only the DATA differs per core.
"""
import sys

sys.path.insert(0, "/opt/trn_rl_repo")

import numpy as np

import concourse.bass as bass
import concourse.bacc as bacc
import concourse.mybir as mybir
import concourse.tile as tile

BF16 = mybir.dt.bfloat16
F32 = mybir.dt.float32
I16 = mybir.dt.int16
np_bf16 = mybir.dt.np(BF16)

CORES = 8
D = 128
NEG = 0.1
N_GRAPHS = 64
N_LAYERS = 3
RANGE = 512          # nodes per PSUM accumulation bank
SRR = 4              # ranges per super-range
CALL_CHUNKS = 32     # 128-edge chunks per dma_gather call (4096 idxs)
NQUEUES = 4
SEG0_TILES = 32      # node tiles in AG segment 0 (small -> early fire)

_cache = {}


def _ceil(a, b):
    return (a + b - 1) // b


def _preprocess(edge_index, batch, n_nodes):
    """Build canonical schedule + per-core data arrays."""
    E = edge_index.shape[1]
    src = np.asarray(edge_index[0], dtype=np.int64)
    dst = np.asarray(edge_index[1], dtype=np.int64)
    NPC = n_nodes // CORES
    ntiles = _ceil(NPC, 128)
    NPCP = ntiles * 128
    s0 = min(SEG0_TILES, max(1, ntiles // 3))
    if ntiles > s0:
        segs = [(0, s0), (s0, ntiles)]
    else:
        segs = [(0, ntiles)]
    seg_rows = [(b - a) * 128 for a, b in segs]
    # buckets: <=32k-row blocks of each segment's gathered buffer (int16 idx)
    seg_blocks = []
    bucket_base = []
    nb_total = 0
    for rs in seg_rows:
        tot = CORES * rs
        nblk = max(1, _ceil(tot, 32768))
        blk = _ceil(tot, nblk)
        seg_blocks.append(blk)
        bucket_base.append(nb_total)
        nb_total += nblk
    NBKT = nb_total

    deg = np.bincount(dst, minlength=n_nodes).astype(np.float32)
    inv_deg = (1.0 / np.maximum(deg, 1.0)).astype(np.float32)

    core = dst // NPC
    dstl = (dst % NPC).astype(np.int64)
    s_rank = src // NPC
    s_l = src % NPC
    bkt = np.zeros(E, np.int64)
    idxval = np.zeros(E, np.int64)
    for i, (a, b) in enumerate(segs):
        lo, hi = a * 128, b * 128
        m = (s_l >= lo) & (s_l < min(hi, NPC))
        row = s_rank[m] * seg_rows[i] + (s_l[m] - lo)
        bkt[m] = bucket_base[i] + row // seg_blocks[i]
        idxval[m] = row % seg_blocks[i]
    nranges = _ceil(NPC, RANGE)
    nsr = _ceil(nranges, SRR)
    sr = (dstl // RANGE) // SRR

    order = np.lexsort((dstl, bkt, sr, core))
    s_src, s_dstl, s_bkt, s_sr, s_core = (
        src[order], dstl[order], bkt[order], sr[order], core[order])

    # counts per (core, sr, bucket)
    seg = ((s_core * nsr + s_sr) * NBKT + s_bkt)
    counts = np.bincount(seg, minlength=CORES * nsr * NBKT).reshape(CORES, nsr, NBKT)
    Kg = _ceil(counts.max(axis=0), 128)  # chunks per (sr, bucket), canonical
    slots_g = Kg * 128                   # [nsr, NBKT]
    slot_base = np.zeros((nsr, NBKT), np.int64)
    flat = slots_g.reshape(-1)
    slot_base.reshape(-1)[1:] = np.cumsum(flat)[:-1]
    TOT = int(flat.sum())
    NCHUNK = TOT // 128

    # per-edge slot position
    seg_sorted = seg  # already sorted by (core, sr, bkt) major
    seg_change = np.empty(E, bool)
    seg_change[0] = True
    seg_change[1:] = seg_sorted[1:] != seg_sorted[:-1]
    seg_start_pos = np.where(seg_change)[0]
    start_per_edge = seg_start_pos[np.cumsum(seg_change) - 1]
    rank = np.arange(E) - start_per_edge
    slot = slot_base[s_sr, s_bkt] + rank  # within-core slot

    # padded per-core arrays; padding idx=-1 -> trailing negatives are
    # dropped by the gather ucode (saves Q7 descriptor-gen time)
    dst_pad = np.full((CORES, TOT), -1, np.int64)
    idx_pad = np.zeros((CORES, TOT), np.int16)
    dst_pad[s_core, slot] = s_dstl
    idx_pad[s_core, slot] = idxval[order].astype(np.int16)

    # canonical chunk windows: min/max real dst over all cores per chunk
    dpc = dst_pad.reshape(CORES, NCHUNK, 128)
    big = np.where(dpc < 0, np.int64(1 << 40), dpc)
    chunk_min = big.min(axis=(0, 2))
    small = np.where(dpc < 0, np.int64(-1), dpc)
    chunk_max = small.max(axis=(0, 2))
    valid_chunk = chunk_max >= 0
    chunk_min = np.where(valid_chunk, chunk_min, 0)
    chunk_max = np.where(valid_chunk, chunk_max, 0)

    # pieces: split [min, max] at RANGE boundaries
    r0 = chunk_min // RANGE
    r1 = chunk_max // RANGE
    max_rel = int((r1 - r0).max()) + 1 if NCHUNK else 1
    piece_r = np.full((NCHUNK, max_rel), -1, np.int64)
    piece_lo = np.zeros((NCHUNK, max_rel), np.int64)
    piece_W = np.zeros((NCHUNK, max_rel), np.int64)
    for rel in range(max_rel):
        r = r0 + rel
        act = valid_chunk & (r <= r1)
        lo = np.maximum(chunk_min, r * RANGE)
        hi = np.minimum(chunk_max, (r + 1) * RANGE - 1)
        piece_r[act, rel] = r[act]
        piece_lo[act, rel] = lo[act]
        piece_W[act, rel] = (hi - lo + 1)[act]
    pw_flat = np.where(piece_r >= 0, piece_W, 0).reshape(-1)
    soff_flat = np.zeros(NCHUNK * max_rel, np.int64)
    soff_flat[1:] = np.cumsum(pw_flat)[:-1]
    piece_soff = soff_flat.reshape(NCHUNK, max_rel)
    SW = int(pw_flat.sum())

    # per-core S matrices [CORES, 128, SW]
    smat = np.zeros((CORES, 128, SW), np.float32)
    e_chunk = slot // 128
    e_row = slot % 128
    e_rel = s_dstl // RANGE - r0[e_chunk]
    e_col = piece_soff[e_chunk, e_rel] + s_dstl - piece_lo[e_chunk, e_rel]
    smat[s_core, e_row, e_col] = inv_deg[dst[order]]

    # idx param wrap: [CORES, 128, TOT//16]; partition p holds stream p%16
    idxw = idx_pad.reshape(CORES, TOT // 16, 16)  # slot-major
    idx_param = np.ascontiguousarray(
        np.tile(idxw.transpose(0, 2, 1), (1, 8, 1)))  # [CORES,128,TOT//16]

    # pooling matrices
    cnt = np.bincount(np.asarray(batch, np.int64), minlength=N_GRAPHS).astype(np.float32)
    inv_cnt = 1.0 / np.maximum(cnt, 1.0)
    gmat = np.zeros((CORES, 128, ntiles * N_GRAPHS), np.float32)
    bnp = np.asarray(batch, np.int64)
    for c in range(CORES):
        loc = bnp[c * NPC:(c + 1) * NPC]
        node = np.arange(NPC)
        t = node // 128
        p = node % 128
        gmat[c, p, t * N_GRAPHS + loc] = inv_cnt[loc]

    sched = dict(
        NPC=NPC, segs=segs, seg_rows=seg_rows, seg_blocks=seg_blocks,
        bucket_base=bucket_base, NBKT=NBKT, nranges=nranges, nsr=nsr, TOT=TOT, SW=SW,
        NCHUNK=NCHUNK, Kg=Kg, slot_base=slot_base,
        piece_r=piece_r, piece_lo=piece_lo, piece_W=piece_W,
        piece_soff=piece_soff, max_rel=max_rel, ntiles=ntiles, NPCP=NPCP,
    )
    data = dict(idx_param=idx_param, smat=smat.astype(np_bf16),
                gmat_param=gmat.astype(np_bf16))
    return sched, data


def _build_nc(sched):
    NPC = sched["NPC"]
    segs = sched["segs"]
    seg_rows = sched["seg_rows"]
    seg_blocks = sched["seg_blocks"]
    bucket_base = sched["bucket_base"]
    NBKT = sched["NBKT"]
    NSEG = len(segs)
    bkt_seg = [None] * NBKT
    bkt_blk = [None] * NBKT
    for _i in range(NSEG):
        _nb = (bucket_base[_i + 1] if _i + 1 < NSEG else NBKT) - bucket_base[_i]
        for _j in range(_nb):
            bkt_seg[bucket_base[_i] + _j] = _i
            bkt_blk[bucket_base[_i] + _j] = _j
    seg_buckets = [[b for b in range(NBKT) if bkt_seg[b] == i] for i in range(NSEG)]
    nranges = sched["nranges"]
    nsr = sched["nsr"]
    TOT = sched["TOT"]
    SW = sched["SW"]
    Kg = sched["Kg"]
    slot_base = sched["slot_base"]
    piece_r = sched["piece_r"]
    piece_lo = sched["piece_lo"]
    piece_W = sched["piece_W"]
    piece_soff = sched["piece_soff"]
    max_rel = sched["max_rel"]
    ntiles = sched["ntiles"]
    NPCP = sched["NPCP"]

    def rwidth(r):
        return min(RANGE, NPC - r * RANGE)

    def sr_ranges(s):
        return list(range(s * SRR, min((s + 1) * SRR, nranges)))

    # per-(sr, bucket) S column spans (contiguous by construction)
    grp_scol = {}
    for s in range(nsr):
        for b in range(NBKT):
            k0 = int(slot_base[s, b]) // 128
            lo, hi = None, None
            for k in range(k0, k0 + int(Kg[s, b])):
                for rel in range(max_rel):
                    if piece_r[k, rel] >= 0 and piece_W[k, rel] > 0:
                        a = int(piece_soff[k, rel])
                        z = a + int(piece_W[k, rel])
                        lo = a if lo is None else min(lo, a)
                        hi = z if hi is None else max(hi, z)
            grp_scol[(s, b)] = (lo, hi) if lo is not None else (0, 0)

    # pieces per (sr, pass, range): for start/stop flags
    def pass_pieces(s, seg_i):
        """{r: [(kk, rel), ...]} for pieces of chunks in seg_i's buckets."""
        out = {}
        for b in seg_buckets[seg_i]:
            k0 = int(slot_base[s, b]) // 128
            for kk in range(k0, k0 + int(Kg[s, b])):
                for rel in range(max_rel):
                    r = int(piece_r[kk, rel])
                    if r >= 0 and piece_W[kk, rel] > 0:
                        out.setdefault(r, []).append((kk, rel))
        return out

    nc = bacc.Bacc(None, target_bir_lowering=False, debug=False,
                   num_devices=CORES, num_swdge_queues=NQUEUES)
    p_x0T = nc.declare_dram_parameter("x0T", [D, NPCP], BF16, isOutput=False)
    p_idx = nc.declare_dram_parameter("idx", [128, TOT // 16], I16, isOutput=False)
    p_smat = nc.declare_dram_parameter("smat", [128, max(SW, 1)], BF16, isOutput=False)
    p_gmat = nc.declare_dram_parameter("gmat", [128, ntiles * N_GRAPHS], BF16, isOutput=False)
    p_wl = nc.declare_dram_parameter("wl", [D, N_LAYERS * D], BF16, isOutput=False)
    p_wr = nc.declare_dram_parameter("wr", [D, N_LAYERS * D], BF16, isOutput=False)
    p_bl = nc.declare_dram_parameter("bl", [D, N_LAYERS], F32, isOutput=False)
    p_wo = nc.declare_dram_parameter("wo", [D, 1], F32, isOutput=False)
    p_bo = nc.declare_dram_parameter("bo", [N_GRAPHS, 1], F32, isOutput=False)
    p_id = nc.declare_dram_parameter("ident", [D, D], BF16, isOutput=False)
    p_out = nc.declare_dram_parameter("out", [N_GRAPHS, 1], F32, isOutput=True)

    h_segs = [[nc.dram_tensor(f"h_seg{j}_{i}", [CORES * seg_rows[i], D], BF16,
                              addr_space="Shared") for i in range(NSEG)]
              for j in range(2)]
    gathers_on = {(j, i): [] for j in range(2) for i in range(NSEG)}

    with tile.TileContext(nc) as tc:
        with (
            tc.tile_pool(name="res", bufs=1) as res,
            tc.tile_pool(name="spool", bufs=3) as spool,
            tc.tile_pool(name="mpool", bufs=6) as mpool,
            tc.tile_pool(name="hpool", bufs=3) as hpool,
            tc.tile_pool(name="pagg", bufs=5, space="PSUM") as pagg,
            tc.tile_pool(name="ph", bufs=2, space="PSUM") as ph,
            tc.tile_pool(name="pmisc", bufs=1, space="PSUM") as pmisc,
            tc.tile_pool(name="dpool", bufs=2, space="DRAM") as dpool,
        ):
            # critical-path loads first: layer-0 H seg0 needs xT seg0 cols + Wl
            seg0_cols = segs[0][1] * 128
            xT = res.tile([D, NPCP], BF16)
            nc.sync.dma_start(out=xT[:, 0:seg0_cols], in_=p_x0T[:, 0:seg0_cols])
            wl_t = res.tile([D, N_LAYERS * D], BF16)
            nc.sync.dma_start(out=wl_t[:, :], in_=p_wl[:, :])

            queue_load = [0] * NQUEUES
            pool_ps = pmisc.tile([128, N_GRAPHS], F32, tag="misc",
                                 name="pool_ps")
            # bf16 partial (Wr + seg0 agg) per node column, resident
            part_t = res.tile([D, NPCP], BF16)

            def emit_h_tiles(lyr, h_loc_v, tlo, thi):
                t = tlo
                while t < thi:
                    nb = min(8, thi - t)
                    h8 = hpool.tile([128, 8, D], BF16, name="h8", tag="h8")
                    for j in range(nb):
                        tt = t + j
                        c0 = tt * 128
                        w = min(128, NPC - c0)
                        psh = ph.tile([128, D], F32, tag="ph", name="psh")
                        nc.tensor.matmul(
                            psh[0:w, :], lhsT=xT[:, c0:c0 + w],
                            rhs=wl_t[:, lyr * D:(lyr + 1) * D],
                            start=True, stop=True)
                        nc.vector.tensor_copy(h8[0:w, j, :], psh[0:w, :])
                    nc.sync.dma_start(out=h_loc_v[:, t:t + nb, :],
                                      in_=h8[:, 0:nb, :])
                    t += nb

            def emit_pool_tiles(tlo, thi):
                for t in range(tlo, thi):
                    c0 = t * 128
                    ptr = ph.tile([128, D], BF16, tag="ph", name="ptr")
                    nc.tensor.transpose(ptr[:, :], xT[:, c0:c0 + 128], id_t[:, :])
                    x3r = hpool.tile([128, D], BF16, name="x3r", tag="x3r")
                    nc.vector.tensor_copy(x3r[:, :], ptr[:, :])
                    nc.tensor.matmul(
                        pool_ps[:, :], lhsT=x3r[:, :],
                        rhs=gmat_t[:, t * N_GRAPHS:(t + 1) * N_GRAPHS],
                        start=(t == 0), stop=(t == ntiles - 1))

            def emit_ag(h_loc, buf, i):
                # trigger from DVE so the Pool sequencer (gather stream) never
                # blocks on the trigger's input-DMA semaphore waits
                a, b = segs[i]
                cc = nc.gpsimd.collective_compute(
                    "AllGather", mybir.AluOpType.bypass,
                    replica_groups=[list(range(CORES))],
                    ins=[h_loc[a * 128:b * 128, :].opt()],
                    outs=[h_segs[buf][i][:, :].opt()],
                )
                for g in gathers_on[(buf, i)]:
                    bass._add_dep_helper(cc.ins, g.ins, True, "AG after old gathers")
                gathers_on[(buf, i)] = []
                return cc

            def emit_group(s, b, layer, cc_cur, last_piece, first_piece, aggs):
                """Gathers + S matmuls for one (sr, bucket) group."""
                k0 = int(slot_base[s, b]) // 128
                K = int(Kg[s, b])
                if K == 0:
                    return
                glo, ghi = grp_scol[(s, b)]
                if ghi > glo:
                    s_t = spool.tile([128, ghi - glo], BF16,
                                     name="sgrp", tag="sgrp")
                    nc.sync.dma_start(out=s_t[:, :], in_=p_smat[:, glo:ghi])
                else:
                    s_t = None
                slo = glo
                ncall = _ceil(K, CALL_CHUNKS)
                base, rem = divmod(K, ncall)
                sizes = [base + (1 if i < rem else 0) for i in range(ncall)]
                k = 0
                for nck in sizes:
                    kstart = k0 + k
                    nidx = nck * 128
                    msg = mpool.tile([128, CALL_CHUNKS, D], BF16, name="msg")
                    colb = kstart * 8  # 128/16
                    si = bkt_seg[b]
                    blk = bkt_blk[b]
                    tot = CORES * seg_rows[si]
                    r0b = blk * seg_blocks[si]
                    r1b = min(r0b + seg_blocks[si], tot)
                    src_ap = h_segs[layer % 2][si][r0b:r1b, :]
                    q = queue_load.index(min(queue_load))
                    queue_load[q] += nidx
                    g = nc.gpsimd.dma_gather(
                        out_ap=msg[:, 0:nck, :],
                        in_ap=src_ap,
                        idxs_ap=idx_t[:, colb:colb + nidx // 16],
                        num_idxs=nidx, num_idxs_reg=nidx,
                        elem_size=D, single_packet=False,
                        queue_num=q,
                    )
                    bass._add_dep_helper(g.ins, cc_cur[si].ins, True, "gather after AG")
                    gathers_on[(layer % 2, si)].append(g)
                    for kk in range(kstart, kstart + nck):
                        for rel in range(max_rel):
                            r = int(piece_r[kk, rel])
                            W = int(piece_W[kk, rel])
                            if r < 0 or W == 0:
                                continue
                            soff = int(piece_soff[kk, rel]) - slo
                            pcol = int(piece_lo[kk, rel]) - r * RANGE
                            nc.tensor.matmul(
                                aggs[r][:, pcol:pcol + W],
                                lhsT=msg[:, kk - kstart, :],
                                rhs=s_t[:, soff:soff + W],
                                start=(first_piece.get(r) == (kk, rel)),
                                stop=(last_piece.get(r) == (kk, rel)),
                            )
                    k += nck

            # ---- layer-0 head: seg0 H + AG0 first ----
            h_loc = dpool.tile([NPCP, D], BF16, name="h_loc", tag="hloc")
            h_loc_v = h_loc.rearrange("(j p) f -> p j f", p=128)
            cc_cur = [None] * NSEG
            emit_h_tiles(0, h_loc_v, segs[0][0], segs[0][1])
            cc_cur[0] = emit_ag(h_loc, 0, 0)
            # seg-1 H + AG next: its completion gates the pass-1 gathers
            idx_t = res.tile([128, TOT // 16], I16)
            nc.sync.dma_start(out=idx_t[:, :], in_=p_idx[:, :])
            if NSEG > 1:
                nc.sync.dma_start(out=xT[:, seg0_cols:], in_=p_x0T[:, seg0_cols:])
                emit_h_tiles(0, h_loc_v, segs[1][0], segs[1][1])
                cc_cur[1] = emit_ag(h_loc, 0, 1)
            # remaining resident loads, off the startup critical path
            wr_t = res.tile([D, N_LAYERS * D], BF16)
            nc.sync.dma_start(out=wr_t[:, :], in_=p_wr[:, :])
            bl_t = res.tile([D, N_LAYERS], F32)
            nc.sync.dma_start(out=bl_t[:, :], in_=p_bl[:, :])
            gmat_t = res.tile([128, ntiles * N_GRAPHS], BF16)
            nc.sync.dma_start(out=gmat_t[:, :], in_=p_gmat[:, :])
            wo_t = res.tile([D, 1], F32)
            nc.sync.dma_start(out=wo_t[:, :], in_=p_wo[:, :])
            bo_t = res.tile([N_GRAPHS, 1], F32)
            nc.sync.dma_start(out=bo_t[:, :], in_=p_bo[:, :])
            id_t = res.tile([D, D], BF16)
            nc.sync.dma_start(out=id_t[:, :], in_=p_id[:, :])
            # warm the msg pool with finite data: padded gather slots are
            # skipped by the ucode, so their (zeroed) S rows must multiply
            # finite bytes, never uninitialized SBUF
            for _ in range(6):
                wmt = mpool.tile([128, CALL_CHUNKS, D], BF16, name="msg")
                nc.vector.memset(wmt[:, :, :], 0.0)

            TPS = (RANGE * SRR) // 128  # node tiles per super-range
            for layer in range(N_LAYERS):
                cc_next = [None] * NSEG
                if layer < N_LAYERS - 1:
                    h_loc = dpool.tile([NPCP, D], BF16, name="h_loc", tag="hloc")
                    h_loc_v = h_loc.rearrange("(j p) f -> p j f", p=128)

                # ---- PASS 0: Wr + segment-0 groups -> bf16 partial ----
                p0_pieces = [pass_pieces(s, 0) for s in range(nsr)]
                p1_pieces = ([pass_pieces(s, 1) for s in range(nsr)]
                             if NSEG > 1 else [{} for _ in range(nsr)])
                p0_aggs = {}
                for s in range(nsr):
                    rs = sr_ranges(s)
                    aggs = {r: pagg.tile([128, RANGE], F32, name="agg", tag="agg")
                            for r in rs}
                    p0_aggs[s] = aggs
                    pieces = p0_pieces[s]
                    last_piece = {r: v[-1] for r, v in pieces.items()}
                    for r in rs:
                        w = rwidth(r)
                        nc.tensor.matmul(
                            aggs[r][:, 0:w], lhsT=wr_t[:, layer * D:(layer + 1) * D],
                            rhs=xT[:, r * RANGE:r * RANGE + w],
                            start=True, stop=(r not in last_piece))
                    for b in seg_buckets[0]:
                        emit_group(s, b, layer, cc_cur, last_piece, {}, aggs)
                    # drain partial (f32 PSUM -> bf16 SBUF)
                    for r in rs:
                        w = rwidth(r)
                        nc.vector.tensor_copy(
                            part_t[:, r * RANGE:r * RANGE + w], aggs[r][:, 0:w])

                # ---- PASS 1: re-add partial + segment-1 groups -> Prelu ----
                for s in range(nsr):
                    rs = sr_ranges(s)
                    aggs = {r: pagg.tile([128, RANGE], F32, name="agg2", tag="agg")
                            for r in rs}
                    pieces = p1_pieces[s]
                    last_piece = {r: v[-1] for r, v in pieces.items()}
                    for r in rs:
                        w = rwidth(r)
                        nc.tensor.matmul(
                            aggs[r][:, 0:w], lhsT=id_t[:, :],
                            rhs=part_t[:, r * RANGE:r * RANGE + w],
                            start=True, stop=(r not in last_piece))
                    if NSEG > 1:
                        for b in seg_buckets[1]:
                            emit_group(s, b, layer, cc_cur, last_piece, {}, aggs)
                    # finalize: leaky-relu(agg + x@Wr + bl) -> new xT columns
                    for r in rs:
                        w = rwidth(r)
                        nc.scalar.activation(
                            xT[:, r * RANGE:r * RANGE + w], aggs[r][:, 0:w],
                            mybir.ActivationFunctionType.Prelu,
                            bias=bl_t[:, layer:layer + 1], scale=1.0, alpha=NEG)
                    # pipelined next-stage work over this super-range's tiles
                    tlo = s * TPS
                    thi = min((s + 1) * TPS, ntiles)
                    if thi > tlo:
                        if layer < N_LAYERS - 1:
                            emit_h_tiles(layer + 1, h_loc_v, tlo, thi)
                            for i, (a, bb) in enumerate(segs):
                                if tlo < bb <= thi:
                                    cc_next[i] = emit_ag(h_loc, (layer + 1) % 2, i)
                        else:
                            emit_pool_tiles(tlo, thi)
                if layer < N_LAYERS - 1:
                    for i in range(NSEG):
                        if cc_next[i] is None:
                            cc_next[i] = emit_ag(h_loc, (layer + 1) % 2, i)
                    cc_cur = cc_next

            # ---- pooling epilogue ----
            pooledT = res.tile([128, N_GRAPHS], F32)
            nc.vector.tensor_copy(pooledT[:, :], pool_ps[:, :])
            fps = pmisc.tile([N_GRAPHS, 1], F32, tag="misc")
            nc.tensor.matmul(fps[:, :], lhsT=pooledT[:, :], rhs=wo_t[:, :],
                             start=True, stop=True)
            partial = res.tile([N_GRAPHS, 1], F32)
            nc.vector.tensor_copy(partial[:, :], fps[:, :])
            ar_in = dpool.tile([N_GRAPHS, 1], F32)
            nc.sync.dma_start(out=ar_in[:, :], in_=partial[:, :])
            ar_out = dpool.tile([N_GRAPHS, 1], F32)
            nc.gpsimd.collective_compute(
                "AllReduce", mybir.AluOpType.add,
                replica_groups=[list(range(CORES))],
                ins=[ar_in[:, :].opt()],
                outs=[ar_out[:, :].opt()],
            )
            summ = res.tile([N_GRAPHS, 1], F32)
            nc.sync.dma_start(out=summ[:, :], in_=ar_out[:, :])
            outt = res.tile([N_GRAPHS, 1], F32)
            nc.scalar.activation(outt[:, :], summ[:, :],
                                 mybir.ActivationFunctionType.Identity,
                                 bias=bo_t[:, 0:1], scale=1.0)
            nc.sync.dma_start(out=p_out[:, :], in_=outt[:, :])

    nc.compile()
    return nc


def _make_in_maps(node_features, Wl, bl, Wr, Wo, bo, sched, data):
    NPC = sched["NPC"]
    NPCP = sched["NPCP"]
    in_maps = []
    wl_h = np.ascontiguousarray(
        np.concatenate([np.asarray(Wl[i]) for i in range(N_LAYERS)], axis=1)
    ).astype(np_bf16)
    wr_h = np.ascontiguousarray(
        np.concatenate([np.asarray(Wr[i]) for i in range(N_LAYERS)], axis=1)
    ).astype(np_bf16)
    bl_h = np.ascontiguousarray(np.asarray(bl, np.float32).T)  # [D, L]
    wo_h = np.asarray(Wo, np.float32).reshape(D, 1)
    bo_h = np.full((N_GRAPHS, 1), np.float32(np.asarray(bo).reshape(-1)[0]))
    id_h = np.eye(D, dtype=np_bf16)
    nf = np.asarray(node_features, np.float32)
    for c in range(CORES):
        x0 = nf[c * NPC:(c + 1) * NPC]
        x0T = np.zeros((D, NPCP), np.float32)
        x0T[:, :NPC] = x0.T
        in_maps.append({
            "x0T": x0T.astype(np_bf16),
            "idx": data["idx_param"][c],
            "smat": data["smat"][c],
            "gmat": data["gmat_param"][c],
            "wl": wl_h, "wr": wr_h, "bl": bl_h,
            "wo": wo_h, "bo": bo_h, "ident": id_h,
        })
    return in_maps


def kernel(node_features, edge_index, batch, Wl, bl, Wr, Wo, bo,
           _trace=False):
    node_features = np.asarray(node_features)
    edge_index = np.asarray(edge_index)
    batch = np.asarray(batch)
    n_nodes = node_features.shape[0]

    key = (n_nodes, edge_index.shape[1],
           hash(edge_index.tobytes()) ^ hash(batch.tobytes()))
    if key in _cache:
        sched, data, nc = _cache[key]
    else:
        sched, data = _preprocess(edge_index, batch, n_nodes)
        nc = _build_nc(sched)
        _cache.clear()
        _cache[key] = (sched, data, nc)

    in_maps = _make_in_maps(node_features, Wl, bl, Wr, Wo, bo, sched, data)

    from concourse import bass_utils
    res = bass_utils.run_bass_kernel_spmd(
        nc, in_maps, core_ids=list(range(CORES)), trace=_trace)
    out = np.asarray(res.results[0]["out"]).reshape(-1)[:N_GRAPHS]
    global last_exec_time_ns
    last_exec_time_ns = res.exec_time_ns
    return out.astype(np.float32)
